# revision 1
# baseline (speedup 1.0000x reference)
"""DeepseekV4-style attention (partial-RoPE LoRA-Q GQA sliding-window) on 8
Trainium2 NeuronCores.

Sharding: core c = 4*b + g handles batch b (of 2) and GQA group g (of 4):
q heads 4g..4g+3, kv head g, the matching column slices of Wqb/Wk/Wv and row
slice of Wo.  Each core computes a partial output `hidden[b]-attention @
Wo[g-slice]`; the host sums the four partials per batch.

All matmuls run in float32r (full PE rate at free-dim >= 256, ~1e-4 rel err).
Layout is "T-layout": Q^T/K^T stored [head_dim, seq] so QK^T and PV need no
transposes; only V needs 16 PE transposes back to natural layout.  Sliding
window + causal masking is applied with gpsimd.affine_select on the exp'd
tiles; the softmax denominator comes from an all-ones matmul that directly
yields a partition-broadcast sum.
"""

import numpy as np
import concourse.bass as bass
import concourse.mybir as mybir
import concourse.tile as tile
from concourse.bass_utils import run_bass_kernel_spmd

F32 = mybir.dt.float32
F32R = mybir.dt.float32r
ACTF = mybir.ActivationFunctionType
ALU = mybir.AluOpType

B, S, D = 2, 2048, 2048
H, KVH, HD = 16, 4, 128
ROT, LORA, WINDOW = 64, 512, 1024
ROPE_BASE = 10000.0
SCALE = HD ** -0.5

HPC = H // KVH          # 4 q heads per core
SB = 512                # free-dim block for matmuls
NSB = S // SB           # 4 seq blocks
KT = D // 128           # 16 contraction tiles over D
ST = S // 128           # 16 seq 128-chunks
N_CORES = 8


def _split_multiwaits(nc):
    """This image's walrus accepts only one embedded SyncWait per instruction;
    split Tile's multi-wait sync_infos into standalone event-semaphore waits."""
    n = 0
    for func in nc.m.functions:
        for bb in func.blocks:
            insts = list(bb.instructions)
            out = []
            changed = False
            for inst in insts:
                si = inst.sync_info
                if si is not None and si.on_wait and len(si.on_wait) > 1:
                    waits = list(si.on_wait)
                    for w in waits[:-1]:
                        ev = mybir.InstEventSemaphore(
                            name=f"{inst.name}_wsplit_{n}", ins=[], outs=[]
                        )
                        ev.engine = inst.engine
                        ev.sync_info = mybir.SyncInfo(on_wait=[w], on_update=[])
                        out.append(ev)
                        n += 1
                    inst.sync_info = mybir.SyncInfo(
                        on_wait=[waits[-1]], on_update=list(si.on_update or [])
                    )
                    changed = True
                out.append(inst)
            if changed:
                bb.instructions = out
    return n


def build_nc(debug=False):
    nc = bass.Bass()
    hid = nc.dram_tensor("hid", [D, S], F32R, kind="ExternalInput")
    wqa = nc.dram_tensor("wqa", [D, LORA], F32R, kind="ExternalInput")
    wqb = nc.dram_tensor("wqb", [LORA, HPC * HD], F32R, kind="ExternalInput")
    wkv = nc.dram_tensor("wkv", [D, 2 * HD], F32R, kind="ExternalInput")
    wo = nc.dram_tensor("wo", [HPC * HD, D], F32R, kind="ExternalInput")
    rcs = nc.dram_tensor("rcs", [128, S], F32R, kind="ExternalInput")
    out = nc.dram_tensor("out", [S, D], F32, kind="ExternalOutput")
    if debug:
        qt_dbg = nc.dram_tensor("qt_dbg", [128, HPC * S], F32R, kind="ExternalOutput")
        kt_dbg = nc.dram_tensor("kt_dbg", [128, S], F32R, kind="ExternalOutput")
        vn_dbg = nc.dram_tensor("vn_dbg", [128, S], F32R, kind="ExternalOutput")
        at_dbg = nc.dram_tensor("at_dbg", [128, HPC * S], F32R, kind="ExternalOutput")
    hidT = hid  # host supplies hidden[b] pre-transposed: [D, S], s contiguous

    with tile.TileContext(nc) as tc:
        with (
            tc.tile_pool(name="cst", bufs=1) as cst,
            tc.tile_pool(name="big", bufs=1) as big,
        ):
            # ---- constants ----
            ropeCC = cst.tile([64, S], F32R, tag="ropeCC")
            nc.sync.dma_start(out=ropeCC[:], in_=rcs[0:64, :])
            ropeSS = cst.tile([64, S], F32R, tag="ropeSS")
            nc.sync.dma_start(out=ropeSS[:], in_=rcs[64:128, :])
            onesf = cst.tile([128, 128], F32, tag="onesf")
            nc.vector.memset(onesf[:], 1.0)
            ones = cst.tile([128, 128], F32R, tag="ones")
            nc.vector.tensor_copy(ones[:], onesf[:])
            identf = cst.tile([128, 128], F32, tag="identf")
            nc.gpsimd.affine_select(
                out=identf[:], in_=onesf[:], pattern=[[1, 128]],
                compare_op=ALU.is_equal, fill=0.0, base=0, channel_multiplier=-1,
            )
            ident = cst.tile([128, 128], F32R, tag="ident")
            nc.vector.tensor_copy(ident[:], identf[:])

            # ---- persistent activations ----
            qT = big.tile([128, HPC * S], F32R, tag="qT")    # per-head Q^T [hd, s]
            kT = big.tile([128, S], F32R, tag="kT")
            vT = big.tile([128, S], F32R, tag="vT")
            vnat = big.tile([128, S], F32R, tag="vnat")      # V rows, 128-chunk t at cols t*128

            def rope_apply(dst, sl, rsl, rp):
                # dst rows 0:64 hold [x1; x2]; rotate in place (T-layout).
                # DVE ops need equal SBUF base partitions, so the half-swap
                # goes through a small SBUF->SBUF DMA.
                swp = rp.tile([64, SB], F32R, tag="swp")
                nc.sync.dma_start(out=swp[0:32, :], in_=dst[32:64, sl])
                nc.sync.dma_start(out=swp[32:64, :], in_=dst[0:32, sl])
                csb = rp.tile([64, SB], F32R, tag="csb")
                nc.vector.tensor_mul(csb[:], dst[0:64, sl], ropeCC[:, rsl])
                tsin = rp.tile([64, SB], F32R, tag="tsin")
                nc.vector.tensor_mul(tsin[:], swp[:], ropeSS[:, rsl])
                nc.vector.tensor_sub(dst[0:32, sl], csb[0:32, :], tsin[0:32, :])
                nc.vector.tensor_add(dst[32:64, sl], csb[32:64, :], tsin[32:64, :])

            with (
                tc.tile_pool(name="tmpA", bufs=1) as tmpA,
                tc.tile_pool(name="hp", bufs=4) as hp,
                tc.tile_pool(name="rp", bufs=2) as rp,
                tc.tile_pool(name="psA", bufs=1, space="PSUM") as psA,
                tc.tile_pool(name="psT", bufs=1, space="PSUM") as psT,
                tc.tile_pool(name="psB", bufs=1, space="PSUM") as psB,
            ):
                # ---- weights for stage 1/2 ----
                wqa_sb = tmpA.tile([128, KT * LORA], F32R, tag="wqa_sb")
                for k in range(KT):
                    nc.sync.dma_start(
                        out=wqa_sb[:, k * LORA:(k + 1) * LORA],
                        in_=wqa[k * 128:(k + 1) * 128, :],
                    )
                wkv_sb = tmpA.tile([128, KT * 256], F32R, tag="wkv_sb")
                for k in range(KT):
                    nc.sync.dma_start(
                        out=wkv_sb[:, k * 256:(k + 1) * 256],
                        in_=wkv[k * 128:(k + 1) * 128, :],
                    )
                wqb_sb = tmpA.tile([128, 4 * HPC * HD], F32R, tag="wqb_sb")
                for k in range(4):
                    nc.sync.dma_start(
                        out=wqb_sb[:, k * 512:(k + 1) * 512],
                        in_=wqb[k * 128:(k + 1) * 128, :],
                    )
                qaT = tmpA.tile([128, 4 * S], F32R, tag="qaT")  # qa^T, m-tile m at cols m*S

                # ---- stage 1: qa^T, k^T, v^T from hidden^T ----
                for sb_i in range(NSB):
                    sl = slice(sb_i * SB, (sb_i + 1) * SB)
                    pq = [
                        psA.tile([128, SB], F32, tag=f"pq{m}", name=f"pq{m}_{sb_i}")
                        for m in range(4)
                    ]
                    pk = psA.tile([128, SB], F32, tag="pk")
                    pv = psA.tile([128, SB], F32, tag="pv")
                    for k in range(KT):
                        ht = hp.tile([128, SB], F32R, tag="ht")
                        nc.sync.dma_start(
                            out=ht[:], in_=hidT[k * 128:(k + 1) * 128, sl]
                        )
                        st, sp = (k == 0), (k == KT - 1)
                        for m in range(4):
                            nc.tensor.matmul(
                                pq[m][:],
                                wqa_sb[:, k * LORA + m * 128: k * LORA + (m + 1) * 128],
                                ht[:], start=st, stop=sp,
                            )
                        nc.tensor.matmul(
                            pk[:], wkv_sb[:, k * 256: k * 256 + 128], ht[:],
                            start=st, stop=sp,
                        )
                        nc.tensor.matmul(
                            pv[:], wkv_sb[:, k * 256 + 128: k * 256 + 256], ht[:],
                            start=st, stop=sp,
                        )
                    for m in range(4):
                        nc.scalar.copy(qaT[:, m * S + sb_i * SB: m * S + (sb_i + 1) * SB],
                                       pq[m][:])
                    nc.scalar.copy(kT[:, sl], pk[:])
                    nc.scalar.copy(vT[:, sl], pv[:])
                    rope_apply(kT, sl, sl, rp)
                    # V natural: PE-transpose the 4 128-chunks of this block
                    for t in range(sb_i * 4, sb_i * 4 + 4):
                        tp = psT.tile([128, 128], F32R, tag="tp")
                        nc.tensor.transpose(tp[:], vT[:, t * 128:(t + 1) * 128], ident[:])
                        nc.vector.tensor_copy(vnat[:, t * 128:(t + 1) * 128], tp[:])

                # ---- stage 2: q^T per head ----
                for sb_i in range(NSB):
                    sl = slice(sb_i * SB, (sb_i + 1) * SB)
                    for h in range(HPC):
                        p2 = psB.tile([128, SB], F32, tag="p2")
                        for k in range(4):
                            nc.tensor.matmul(
                                p2[:],
                                wqb_sb[:, k * 512 + h * 128: k * 512 + (h + 1) * 128],
                                qaT[:, k * S + sb_i * SB: k * S + (sb_i + 1) * SB],
                                start=(k == 0), stop=(k == 3),
                            )
                        nc.scalar.copy(qT[:, h * S + sb_i * SB: h * S + (sb_i + 1) * SB],
                                       p2[:])
                        rope_apply(qT, slice(h * S + sb_i * SB, h * S + (sb_i + 1) * SB),
                                   sl, rp)

            if debug:
                nc.sync.dma_start(out=qt_dbg[:], in_=qT[:])
                nc.sync.dma_start(out=kt_dbg[:], in_=kT[:])
                nc.sync.dma_start(out=vn_dbg[:], in_=vnat[:])

            # ---- stage 3: attention ----
            with tc.tile_pool(name="bigB", bufs=1) as bigB:
                attnT = bigB.tile([128, HPC * S], F32R, tag="attnT")
                with (
                    tc.tile_pool(name="ex", bufs=4) as ex,
                    tc.tile_pool(name="rc", bufs=2) as rc,
                    tc.tile_pool(name="psL", bufs=2, space="PSUM") as psL,
                    tc.tile_pool(name="psO", bufs=2, space="PSUM") as psO,
                    tc.tile_pool(name="psD", bufs=2, space="PSUM") as psD,
                ):
                    for h in range(HPC):
                        for qb in range(NSB):
                            q0 = qb * SB
                            qsl = slice(h * S + q0, h * S + q0 + SB)
                            kt_lo = max(0, q0 - WINDOW + 1) // 128
                            kt_hi = q0 // 128 + 3
                            po = psO.tile([128, SB], F32, tag="po")
                            pd = psD.tile([128, SB], F32, tag="pd")
                            for kt in range(kt_lo, kt_hi + 1):
                                dp = kt * 128 - q0
                                pl = psL.tile([128, SB], F32, tag="pl")
                                nc.tensor.matmul(
                                    pl[:], kT[:, kt * 128:(kt + 1) * 128], qT[:, qsl],
                                    start=True, stop=True,
                                )
                                e = ex.tile([128, SB], F32R, tag="e")
                                nc.scalar.activation(e[:], pl[:], ACTF.Exp, scale=SCALE)
                                if dp >= 0:
                                    # causal edge: keep j - i - dp >= 0
                                    nc.gpsimd.affine_select(
                                        out=e[:], in_=e[:], pattern=[[1, SB]],
                                        compare_op=ALU.is_ge, fill=0.0,
                                        base=-dp, channel_multiplier=-1,
                                    )
                                elif dp <= SB - WINDOW:
                                    # window edge: keep (q0+j)-(k0+i) = j-i-dp
                                    # < WINDOW, i.e. WINDOW-1+dp + i - j >= 0
                                    nc.gpsimd.affine_select(
                                        out=e[:], in_=e[:], pattern=[[-1, SB]],
                                        compare_op=ALU.is_ge, fill=0.0,
                                        base=WINDOW - 1 + dp, channel_multiplier=1,
                                    )
                                st, sp = (kt == kt_lo), (kt == kt_hi)
                                nc.tensor.matmul(
                                    po[:], vnat[:, kt * 128:(kt + 1) * 128], e[:],
                                    start=st, stop=sp,
                                )
                                nc.tensor.matmul(pd[:], ones[:], e[:], start=st, stop=sp)
                            rec = rc.tile([128, SB], F32, tag="rec")
                            nc.vector.reciprocal(rec[:], pd[:])
                            nc.vector.tensor_mul(attnT[:, qsl], po[:], rec[:])

                if debug:
                    nc.sync.dma_start(out=at_dbg[:], in_=attnT[:])

                # ---- stage 4: output projection (partial over this head group) ----
                with (
                    tc.tile_pool(name="tmpB", bufs=1) as tmpB,
                    tc.tile_pool(name="od", bufs=2) as od,
                    tc.tile_pool(name="psW", bufs=4, space="PSUM") as psW,
                ):
                    wo_sb = tmpB.tile([128, HPC * D], F32R, tag="wo_sb")
                    for h in range(HPC):
                        nc.sync.dma_start(
                            out=wo_sb[:, h * D:(h + 1) * D],
                            in_=wo[h * 128:(h + 1) * 128, :],
                        )
                    for t in range(ST):
                        ot = od.tile([128, D], F32, tag="ot")
                        for n in range(4):
                            pw = psW.tile([128, SB], F32, tag="pw")
                            for h in range(HPC):
                                nc.tensor.matmul(
                                    pw[:],
                                    attnT[:, h * S + t * 128: h * S + (t + 1) * 128],
                                    wo_sb[:, h * D + n * SB: h * D + (n + 1) * SB],
                                    start=(h == 0), stop=(h == HPC - 1),
                                )
                            nc.scalar.copy(ot[:, n * SB:(n + 1) * SB], pw[:])
                        nc.sync.dma_start(
                            out=out[t * 128:(t + 1) * 128, :], in_=ot[:]
                        )
    _split_multiwaits(nc)
    return nc


_NC = None


def _get_nc():
    global _NC
    if _NC is None:
        _NC = build_nc()
    return _NC


def _make_in_maps(hidden, position_ids, Wqa, Wqb, Wk, Wv, Wo):
    hidden = np.asarray(hidden, dtype=np.float32)
    position_ids = np.asarray(position_ids)
    Wqa = np.ascontiguousarray(np.asarray(Wqa, dtype=np.float32))
    Wqb = np.asarray(Wqb, dtype=np.float32)
    Wk = np.asarray(Wk, dtype=np.float32)
    Wv = np.asarray(Wv, dtype=np.float32)
    Wo = np.asarray(Wo, dtype=np.float32)

    inv_freq = 1.0 / (ROPE_BASE ** (np.arange(0, ROT, 2, dtype=np.float32) / ROT))
    in_maps = []
    for c in range(N_CORES):
        b, g = c // KVH, c % KVH
        pos = position_ids[b].astype(np.float32)
        freqs = pos[:, None] * inv_freq[None, :]        # [S, 32]
        cosT = np.cos(freqs).T.astype(np.float32)       # [32, S]
        sinT = np.sin(freqs).T.astype(np.float32)
        rcs = np.concatenate([cosT, cosT, sinT, sinT], axis=0)  # [128, S]
        in_maps.append({
            "hid": np.ascontiguousarray(hidden[b].T),
            "wqa": Wqa,
            "wqb": np.ascontiguousarray(Wqb[:, g * HPC * HD:(g + 1) * HPC * HD]),
            "wkv": np.ascontiguousarray(
                np.concatenate(
                    [Wk[:, g * HD:(g + 1) * HD], Wv[:, g * HD:(g + 1) * HD]], axis=1
                )
            ),
            "wo": np.ascontiguousarray(Wo[g * HPC * HD:(g + 1) * HPC * HD, :]),
            "rcs": np.ascontiguousarray(rcs),
        })
    return in_maps


def _run(inputs, trace=False):
    nc = _get_nc()
    in_maps = _make_in_maps(**inputs)
    res = run_bass_kernel_spmd(nc, in_maps, list(range(N_CORES)), trace=trace)
    out = np.zeros((B, S, D), dtype=np.float32)
    for c in range(N_CORES):
        out[c // KVH] += res.results[c]["out"]
    return out, res


def kernel(**inputs) -> np.ndarray:
    return _run(inputs, trace=False)[0]



# revision 8
# speedup vs baseline: 1.1867x; 1.1867x over previous
"""DeepseekV4-style attention (partial-RoPE LoRA-Q GQA sliding-window) on 8
Trainium2 NeuronCores.

Sharding: core c = 4*b + g handles batch b (of 2) and GQA group g (of 4):
q heads 4g..4g+3, kv head g, the matching column slices of Wq/Wk/Wv and row
slice of Wo.  Each core computes a partial output; the host sums the four
partials per batch.

v1 design (vs the fp32r baseline):
- All matmul operands bf16 (host-converted); PSUM accumulation stays f32.
- LoRA folded on host: Wq = Wqa @ Wqb[:, group] so the Q projection is a
  single GEMM and the duplicated qa stage disappears.
- Single-pass pipeline over 512-seq blocks: fused QKV projection -> rope ->
  attention (2 heads interleaved, PV one kt-step behind QK so the PE never
  waits on exp/mask) -> output projection, all within 8 PSUM banks.
- Softmax denominator: e-tiles accumulated on the Vector engine (esum += e),
  then ONE all-ones matmul per (head, block) instead of one per kt tile.
- reciprocal_approx_fast for 1/denominator.
"""

import numpy as np
import concourse.bass as bass
import concourse.mybir as mybir
import concourse.tile as tile
from concourse.bass_utils import run_bass_kernel_spmd

F32 = mybir.dt.float32
BF16 = mybir.dt.bfloat16
ACTF = mybir.ActivationFunctionType
ALU = mybir.AluOpType

B, S, D = 2, 2048, 2048
H, KVH, HD = 16, 4, 128
ROT, LORA, WINDOW = 64, 512, 1024
ROPE_BASE = 10000.0
SCALE = HD ** -0.5

HPC = H // KVH          # 4 q heads per core
SB = 512                # free-dim block
NSB = S // SB           # 4 seq blocks
KT = D // 128           # 16 contraction tiles over D
ST = S // 128           # 16 seq 128-chunks
N_CORES = 8


def _split_multiwaits(nc):
    """This image's walrus accepts only one embedded SyncWait per instruction;
    split Tile's multi-wait sync_infos into standalone event-semaphore waits."""
    n = 0
    for func in nc.m.functions:
        for bb in func.blocks:
            insts = list(bb.instructions)
            out = []
            changed = False
            for inst in insts:
                si = inst.sync_info
                if si is not None and si.on_wait and len(si.on_wait) > 1:
                    waits = list(si.on_wait)
                    for w in waits[:-1]:
                        ev = mybir.InstEventSemaphore(
                            name=f"{inst.name}_wsplit_{n}", ins=[], outs=[]
                        )
                        ev.engine = inst.engine
                        ev.sync_info = mybir.SyncInfo(on_wait=[w], on_update=[])
                        out.append(ev)
                        n += 1
                    inst.sync_info = mybir.SyncInfo(
                        on_wait=[waits[-1]], on_update=list(si.on_update or [])
                    )
                    changed = True
                out.append(inst)
            if changed:
                bb.instructions = out
    return n


def build_nc(debug=False):
    nc = bass.Bass()
    hid = nc.dram_tensor("hid", [D, S], BF16, kind="ExternalInput")
    wq = nc.dram_tensor("wq", [D, HPC * HD], BF16, kind="ExternalInput")
    wkv = nc.dram_tensor("wkv", [D, 2 * HD], BF16, kind="ExternalInput")
    wo = nc.dram_tensor("wo", [HPC * HD, D], BF16, kind="ExternalInput")
    rcs = nc.dram_tensor("rcs", [128, S], BF16, kind="ExternalInput")
    out = nc.dram_tensor("out", [S, D], BF16, kind="ExternalOutput")
    if debug:
        qt_dbg = nc.dram_tensor("qt_dbg", [128, HPC * S], BF16, kind="ExternalOutput")
        kt_dbg = nc.dram_tensor("kt_dbg", [128, S], BF16, kind="ExternalOutput")
        vn_dbg = nc.dram_tensor("vn_dbg", [128, S], BF16, kind="ExternalOutput")
        at_dbg = nc.dram_tensor("at_dbg", [128, HPC * S], BF16, kind="ExternalOutput")

    with tile.TileContext(nc) as tc:
        with (
            tc.tile_pool(name="cst", bufs=1) as cst,
            tc.tile_pool(name="big", bufs=1) as big,
            tc.tile_pool(name="hp", bufs=2) as hp,
            tc.tile_pool(name="rp", bufs=2) as rp,
            tc.tile_pool(name="ex", bufs=4) as ex,
            tc.tile_pool(name="es", bufs=1) as es,
            tc.tile_pool(name="rcp", bufs=2) as rcp,
            tc.tile_pool(name="od", bufs=2) as od,
            tc.tile_pool(name="psP", bufs=1, space="PSUM") as psP,
            tc.tile_pool(name="psL", bufs=1, space="PSUM") as psL,
            tc.tile_pool(name="psO", bufs=1, space="PSUM") as psO,
            tc.tile_pool(name="psD", bufs=1, space="PSUM") as psD,
            tc.tile_pool(name="psT", bufs=1, space="PSUM") as psT,
        ):
            # ---- weights: per-kt tiles, DMAs interleaved for fast rampup ----
            wkv_t = [cst.tile([128, 256], BF16, tag=f"wkv{k}", name=f"wkv{k}") for k in range(KT)]
            wq_t = [cst.tile([128, 512], BF16, tag=f"wq{k}", name=f"wq{k}") for k in range(KT)]
            hb0 = hp.tile([128, KT * SB], BF16, tag="hb", name="hb_0")
            for k in range(KT):
                nc.sync.dma_start(out=wkv_t[k][:], in_=wkv[k * 128:(k + 1) * 128, :])
                nc.sync.dma_start(out=hb0[:, k * SB:(k + 1) * SB],
                                  in_=hid[k * 128:(k + 1) * 128, 0:SB])
                nc.sync.dma_start(out=wq_t[k][:], in_=wq[k * 128:(k + 1) * 128, :])

            # ---- constants ----
            ropeCC = cst.tile([64, S], BF16, tag="ropeCC")
            nc.sync.dma_start(out=ropeCC[:], in_=rcs[0:64, :])
            ropeSS = cst.tile([64, S], BF16, tag="ropeSS")
            nc.sync.dma_start(out=ropeSS[:], in_=rcs[64:128, :])
            onesf = cst.tile([128, 128], F32, tag="onesf")
            nc.vector.memset(onesf[:], 1.0)
            ones = cst.tile([128, 128], BF16, tag="ones")
            nc.vector.tensor_copy(ones[:], onesf[:])
            identf = cst.tile([128, 128], F32, tag="identf")
            nc.gpsimd.affine_select(
                out=identf[:], in_=onesf[:], pattern=[[1, 128]],
                compare_op=ALU.is_equal, fill=0.0, base=0, channel_multiplier=-1,
            )
            ident = cst.tile([128, 128], BF16, tag="ident")
            nc.vector.tensor_copy(ident[:], identf[:])

            wo_t = [cst.tile([128, D], BF16, tag=f"wo{h}", name=f"wo{h}") for h in range(HPC)]
            for h in range(HPC):
                nc.sync.dma_start(out=wo_t[h][:], in_=wo[h * 128:(h + 1) * 128, :])

            # ---- persistent activations ----
            qT = big.tile([128, HPC * S], BF16, tag="qT")    # per-head Q^T [hd, s]
            kT = big.tile([128, S], BF16, tag="kT")
            vT = big.tile([128, S], BF16, tag="vT")
            vnat = big.tile([128, S], BF16, tag="vnat")      # V rows, chunk t at cols t*128

            def rope_apply(dst, sl, rsl):
                # dst rows 0:64 hold [x1; x2]; rotate in place (T-layout).
                # DVE ops need equal SBUF base partitions, so the half-swap
                # goes through a small SBUF->SBUF DMA.
                swp = rp.tile([64, SB], BF16, tag="swp")
                nc.sync.dma_start(out=swp[0:32, :], in_=dst[32:64, sl])
                nc.sync.dma_start(out=swp[32:64, :], in_=dst[0:32, sl])
                csb = rp.tile([64, SB], BF16, tag="csb")
                nc.vector.tensor_mul(csb[:], dst[0:64, sl], ropeCC[:, rsl])
                tsin = rp.tile([64, SB], BF16, tag="tsin")
                nc.vector.tensor_mul(tsin[:], swp[:], ropeSS[:, rsl])
                nc.vector.tensor_sub(dst[0:32, sl], csb[0:32, :], tsin[0:32, :])
                nc.vector.tensor_add(dst[32:64, sl], csb[32:64, :], tsin[32:64, :])

            # pipeline over seq blocks: proj(sb) -> rope -> attn(qb=sb) -> out(sb)
            hb_tiles = {0: hb0}
            for sb_i in range(NSB):
                sl = slice(sb_i * SB, (sb_i + 1) * SB)
                hb = hb_tiles[sb_i]

                # ---- stage 1: fused QKV projection, 3 waves of 2 psum banks ----
                # wave 0: k, v; wave 1: q0, q1; wave 2: q2, q3
                waves = [
                    [("k", None), ("v", None)],
                    [("q", 0), ("q", 1)],
                    [("q", 2), ("q", 3)],
                ]
                for wv in waves:
                    pg = [psP.tile([128, SB], F32, tag=f"pg{i}",
                                   name=f"pg{i}_{sb_i}_{wv[0][0]}{wv[0][1]}")
                          for i in range(2)]
                    for k in range(KT):
                        for i, (kind, idx) in enumerate(wv):
                            if kind == "k":
                                w_ap = wkv_t[k][:, 0:128]
                            elif kind == "v":
                                w_ap = wkv_t[k][:, 128:256]
                            else:
                                w_ap = wq_t[k][:, idx * 128:(idx + 1) * 128]
                            nc.tensor.matmul(
                                pg[i][:], w_ap, hb[:, k * SB:(k + 1) * SB],
                                start=(k == 0), stop=(k == KT - 1),
                            )
                    for i, (kind, idx) in enumerate(wv):
                        if kind == "k":
                            nc.scalar.copy(kT[:, sl], pg[i][:])
                            rope_apply(kT, sl, sl)
                        elif kind == "v":
                            nc.scalar.copy(vT[:, sl], pg[i][:])
                        else:
                            dsl = slice(idx * S + sb_i * SB, idx * S + (sb_i + 1) * SB)
                            nc.scalar.copy(qT[:, dsl], pg[i][:])
                            rope_apply(qT, dsl, sl)

                # V natural: PE-transpose the 4 128-chunks of this block
                for t in range(sb_i * 4, sb_i * 4 + 4):
                    tp = psT.tile([128, 128], BF16, tag="tp")
                    nc.tensor.transpose(tp[:], vT[:, t * 128:(t + 1) * 128], ident[:])
                    nc.vector.tensor_copy(vnat[:, t * 128:(t + 1) * 128], tp[:])

                # prefetch next block's hidden tiles (gpsimd queue: issues as
                # soon as the previous block's masking is done, well before
                # stage 1 of the next block needs them)
                if sb_i + 1 < NSB:
                    nsl = slice((sb_i + 1) * SB, (sb_i + 2) * SB)
                    nhb = hp.tile([128, KT * SB], BF16, tag="hb",
                                  name=f"hb_{sb_i + 1}")
                    hb_tiles[sb_i + 1] = nhb
                    for k in range(KT):
                        nc.gpsimd.dma_start(
                            out=nhb[:, k * SB:(k + 1) * SB],
                            in_=hid[k * 128:(k + 1) * 128, nsl])

                # ---- stage 3: attention for qb = sb_i, two heads interleaved ----
                q0 = sb_i * SB
                kt_lo = max(0, q0 - WINDOW + 1) // 128
                kt_hi = q0 // 128 + 3
                attn = rcp.tile([128, HPC * SB], BF16, tag="attn")
                for hp2 in ((0, 1), (2, 3)):
                    po = {}
                    esum = {}
                    e_cur = {}
                    for j, h in enumerate(hp2):
                        po[h] = psO.tile([128, SB], F32, tag=f"po{j}",
                                         name=f"po{j}_{sb_i}")
                        esum[h] = es.tile([128, SB], BF16, tag=f"es{j}",
                                          name=f"es{j}_{sb_i}")
                    # software-pipelined: QK at kt, PV at kt-1
                    for kt in range(kt_lo, kt_hi + 2):
                        for j, h in enumerate(hp2):
                            if kt <= kt_hi:
                                qsl = slice(h * S + q0, h * S + q0 + SB)
                                dp = kt * 128 - q0
                                pl = psL.tile([128, SB], F32, tag=f"pl{j}",
                                              name=f"pl{j}_{sb_i}_{kt}")
                                nc.tensor.matmul(
                                    pl[:], kT[:, kt * 128:(kt + 1) * 128],
                                    qT[:, qsl], start=True, stop=True,
                                )
                                e = ex.tile([128, SB], BF16, tag="e")
                                nc.scalar.activation(e[:], pl[:], ACTF.Exp,
                                                     scale=SCALE)
                                if dp >= 0:
                                    nc.gpsimd.affine_select(
                                        out=e[:], in_=e[:], pattern=[[1, SB]],
                                        compare_op=ALU.is_ge, fill=0.0,
                                        base=-dp, channel_multiplier=-1,
                                    )
                                elif dp <= -(WINDOW - SB + 128):
                                    nc.gpsimd.affine_select(
                                        out=e[:], in_=e[:], pattern=[[-1, SB]],
                                        compare_op=ALU.is_ge, fill=0.0,
                                        base=WINDOW - 1 + dp, channel_multiplier=1,
                                    )
                                if kt == kt_lo:
                                    nc.vector.tensor_copy(esum[h][:], e[:])
                                else:
                                    nc.vector.tensor_add(esum[h][:], esum[h][:],
                                                         e[:])
                                e_cur[h] = (kt, e)
                            if kt > kt_lo:
                                pkt, pe = e_prev[h]
                                nc.tensor.matmul(
                                    po[h][:], vnat[:, pkt * 128:(pkt + 1) * 128],
                                    pe[:], start=(pkt == kt_lo),
                                    stop=(pkt == kt_hi),
                                )
                        e_prev = dict(e_cur)
                    for j, h in enumerate(hp2):
                        pd = psD.tile([128, SB], F32, tag="pd",
                                      name=f"pd_{sb_i}_{h}")
                        nc.tensor.matmul(pd[:], ones[:], esum[h][:],
                                         start=True, stop=True)
                        rec = rcp.tile([128, SB], F32, tag="rec")
                        nc.vector.reciprocal(rec[:], pd[:])
                        nc.vector.tensor_mul(attn[:, h * SB:(h + 1) * SB],
                                             po[h][:], rec[:])

                if debug:
                    for h in range(HPC):
                        nc.sync.dma_start(
                            out=at_dbg[:, h * S + sb_i * SB:h * S + (sb_i + 1) * SB],
                            in_=attn[:, h * SB:(h + 1) * SB])

                # ---- stage 4: output projection for rows in this block ----
                for ti, t in enumerate(range(sb_i * 4, sb_i * 4 + 4)):
                    ot = od.tile([128, D], BF16, tag="ot")
                    for n in range(4):
                        pw = psL.tile([128, SB], F32, tag=f"pl{n % 2}",
                                      name=f"pw_{t}_{n}")
                        for h in range(HPC):
                            nc.tensor.matmul(
                                pw[:],
                                attn[:, h * SB + ti * 128:h * SB + (ti + 1) * 128],
                                wo_t[h][:, n * SB:(n + 1) * SB],
                                start=(h == 0), stop=(h == HPC - 1),
                            )
                        nc.scalar.copy(ot[:, n * SB:(n + 1) * SB], pw[:])
                    nc.sync.dma_start(out=out[t * 128:(t + 1) * 128, :], in_=ot[:])

            if debug:
                nc.sync.dma_start(out=qt_dbg[:], in_=qT[:])
                nc.sync.dma_start(out=kt_dbg[:], in_=kT[:])
                nc.sync.dma_start(out=vn_dbg[:], in_=vnat[:])
    _split_multiwaits(nc)
    return nc


_NC = None


def _get_nc():
    global _NC
    if _NC is None:
        _NC = build_nc()
    return _NC


def _make_in_maps(hidden, position_ids, Wqa, Wqb, Wk, Wv, Wo):
    import ml_dtypes
    bf16 = ml_dtypes.bfloat16
    hidden = np.asarray(hidden, dtype=np.float32)
    position_ids = np.asarray(position_ids)
    Wqa = np.asarray(Wqa, dtype=np.float32)
    Wqb = np.asarray(Wqb, dtype=np.float32)
    Wk = np.asarray(Wk, dtype=np.float32)
    Wv = np.asarray(Wv, dtype=np.float32)
    Wo = np.asarray(Wo, dtype=np.float32)

    inv_freq = 1.0 / (ROPE_BASE ** (np.arange(0, ROT, 2, dtype=np.float32) / ROT))
    hidT = [np.ascontiguousarray(hidden[b].T).astype(bf16) for b in range(B)]
    Wq_full = Wqa @ Wqb  # [D, H*HD] folded LoRA
    in_maps = []
    for c in range(N_CORES):
        b, g = c // KVH, c % KVH
        pos = position_ids[b].astype(np.float32)
        freqs = pos[:, None] * inv_freq[None, :]        # [S, 32]
        cosT = np.cos(freqs).T.astype(np.float32)       # [32, S]
        sinT = np.sin(freqs).T.astype(np.float32)
        rcs = np.concatenate([cosT, cosT, sinT, sinT], axis=0)  # [128, S]
        in_maps.append({
            "hid": hidT[b],
            "wq": np.ascontiguousarray(
                Wq_full[:, g * HPC * HD:(g + 1) * HPC * HD]).astype(bf16),
            "wkv": np.ascontiguousarray(
                np.concatenate(
                    [Wk[:, g * HD:(g + 1) * HD], Wv[:, g * HD:(g + 1) * HD]], axis=1
                )).astype(bf16),
            "wo": np.ascontiguousarray(
                Wo[g * HPC * HD:(g + 1) * HPC * HD, :]).astype(bf16),
            "rcs": np.ascontiguousarray(rcs).astype(bf16),
        })
    return in_maps


def _run(inputs, trace=False):
    nc = _get_nc()
    in_maps = _make_in_maps(**inputs)
    res = run_bass_kernel_spmd(nc, in_maps, list(range(N_CORES)), trace=trace)
    out = np.zeros((B, S, D), dtype=np.float32)
    for c in range(N_CORES):
        out[c // KVH] += res.results[c]["out"].astype(np.float32)
    return out, res


def kernel(**inputs) -> np.ndarray:
    return _run(inputs, trace=False)[0]
